# revision 27
# baseline (speedup 1.0000x reference)
"""Multi-head attention (B=2, S=2048, D=1024, H=16) on 8 Trainium2 cores.

Sharding: 2 heads per core (tensor-parallel on H). Each core computes its
2 heads' QKV projections, attention, and a partial output projection
(the 128 columns of the concat dim it owns); the host sums the 8 partial
outputs and adds the output bias.

v2: all PE operands in bf16 (no f32r cast pass), bf16 partial-output
writes, large x DMAs. Device dataflow per (batch, head):
  qT/kT = W x^T         [64, S]   (bf16 x bf16 -> f32 psum -> +bias -> bf16)
  vT    = Wv xv^T       -> PE-transpose -> v natural [S, 64] (+ones col)
  sT    = kT^T q        [t-block 128, s 512] transposed scores (psum)
  expS  = exp(sT/8)     (ScalarE, psum -> sbuf bf16)
  o~T/Z = [v|1]^T expS  [65, s]  (P@V with ones column -> row 64 = Z)
  oT    = o~T * (1/Z)   (recip + PE outer-product broadcast + DVE mul)
  y_c   = oT^T Wo_c^T   [s-block 128, 1024] partial output (psum->sbuf bf16->DRAM)
"""

import os
import numpy as np
import ml_dtypes

B, S, D, H = 2, 2048, 1024, 16
HD = D // H          # 64
NCORES = 8
HPC = H // NCORES    # 2 heads per core
P = 128
SC = 512             # s-chunk width
NSC = S // SC        # 4
NKB = D // P         # 8 contraction blocks for projections
NTB = S // P         # 16 t-blocks
SH = S // 2          # half-row width for x tiles

_BF16 = ml_dtypes.bfloat16

_nc_cache = {}
_runner_cache = {}

# timing-probe knobs (correctness-breaking; for bottleneck attribution only)
_PROBE = os.environ.get("KPROBE", "")


def build_nc(loop_k: int = 1):
    """Build (and cache) the per-core Bass module. loop_k>1 wraps the body
    in a hardware loop for timing measurements."""
    key = (loop_k, _PROBE)
    if key in _nc_cache:
        return _nc_cache[key]

    import concourse.bass as bass
    import concourse.mybir as mybir
    import concourse.tile as tile
    from concourse import bacc
    from concourse.masks import make_identity
    from contextlib import ExitStack

    f32 = mybir.dt.float32
    bf16 = mybir.dt.bfloat16
    AF = mybir.ActivationFunctionType

    nc = bacc.Bacc("TRN2", target_bir_lowering=False)

    xqT = nc.dram_tensor("xqT", [B, D, S], bf16, kind="ExternalInput")
    xkT = nc.dram_tensor("xkT", [B, D, S], bf16, kind="ExternalInput")
    xvT = nc.dram_tensor("xvT", [B, D, S], bf16, kind="ExternalInput")
    wq = nc.dram_tensor("wq", [D, P], bf16, kind="ExternalInput")
    wk = nc.dram_tensor("wk", [D, P], bf16, kind="ExternalInput")
    wv = nc.dram_tensor("wv", [D, P], bf16, kind="ExternalInput")
    bq = nc.dram_tensor("bq", [P, 1], f32, kind="ExternalInput")
    bk = nc.dram_tensor("bk", [P, 1], f32, kind="ExternalInput")
    bv = nc.dram_tensor("bv", [P, 1], f32, kind="ExternalInput")
    wo = nc.dram_tensor("wo", [P, D], bf16, kind="ExternalInput")
    ypart = nc.dram_tensor("ypart", [B, S, D], bf16, kind="ExternalOutput")

    with tile.TileContext(nc) as tc:
        with ExitStack() as ctx:
            const = ctx.enter_context(tc.tile_pool(name="const", bufs=1))
            xin = ctx.enter_context(tc.tile_pool(name="xin", bufs=3))
            qkv = ctx.enter_context(tc.tile_pool(name="qkv", bufs=2))
            otp = ctx.enter_context(tc.tile_pool(name="otp", bufs=3))
            vap = ctx.enter_context(tc.tile_pool(name="vap", bufs=2))
            expp = ctx.enter_context(tc.tile_pool(name="expp", bufs=4))
            smalls = ctx.enter_context(tc.tile_pool(name="smalls", bufs=4))
            yout = ctx.enter_context(tc.tile_pool(name="yout", bufs=3))
            # PSUM budget (8 banks): pp = score tiles ([128,1024] f32 = 2
            # banks) x2 bufs = 4; pph = P@V accumulators ([65,512] = 1 bank)
            # x2 = 2; ppw = bc/outproj work tiles x1 = 1; ppm = proj x1 = 1.
            pp = ctx.enter_context(tc.tile_pool(name="pp", bufs=2, space="PSUM"))
            pph = ctx.enter_context(tc.tile_pool(name="pph", bufs=2, space="PSUM"))
            ppw = ctx.enter_context(tc.tile_pool(name="ppw", bufs=1, space="PSUM"))
            ppm = ctx.enter_context(tc.tile_pool(name="ppm", bufs=1, space="PSUM"))

            # ---- constants (outside the timing loop) ----
            wq_sb = const.tile([P, NKB, P], bf16, tag="wq")
            wk_sb = const.tile([P, NKB, P], bf16, tag="wk")
            wv_sb = const.tile([P, NKB, P], bf16, tag="wv")
            nc.sync.dma_start(wq_sb[:], wq.ap().rearrange("(a p) e -> p a e", p=P))
            nc.sync.dma_start(wk_sb[:], wk.ap().rearrange("(a p) e -> p a e", p=P))
            nc.sync.dma_start(wv_sb[:], wv.ap().rearrange("(a p) e -> p a e", p=P))
            wo_sb = const.tile([P, D], bf16, tag="wo")
            nc.sync.dma_start(wo_sb[:], wo[:, :])
            bq_sb = const.tile([P, 1], f32, tag="bq")
            bk_sb = const.tile([P, 1], f32, tag="bk")
            bv_sb = const.tile([P, 1], f32, tag="bv")
            nc.sync.dma_start(bq_sb[:], bq[:, :])
            nc.sync.dma_start(bk_sb[:], bk[:, :])
            nc.sync.dma_start(bv_sb[:], bv[:, :])
            ident_f32 = const.tile([P, P], f32, tag="identf")
            make_identity(nc, ident_f32[:])
            ident = const.tile([P, P], bf16, tag="ident")
            nc.vector.tensor_copy(ident[:], ident_f32[:])
            ones_f32 = const.tile([P, HD], f32, tag="onesf")
            nc.vector.memset(ones_f32[:], 1.0)
            ones_bf = const.tile([P, HD], bf16, tag="onesb")
            nc.vector.tensor_copy(ones_bf[:], ones_f32[:])
            ones_row = ones_bf[0:1, 0:HD]

            from collections import deque

            xmaps = {
                id(xqT): xqT.ap().rearrange("b (a p) s -> b p a s", p=P),
                id(xkT): xkT.ap().rearrange("b (a p) s -> b p a s", p=P),
                id(xvT): xvT.ap().rearrange("b (a p) s -> b p a s", p=P),
            }

            def body():
                # Background work queue of (key, generator) pumped between
                # attention steps so PE/DVE gaps absorb DMA-issue, projection
                # and transpose work while ACT streams exps.  Correctness
                # does NOT depend on pump pacing: ensure(key) force-drains
                # the queue (in order) until the keyed item has been fully
                # emitted.  Trace order defines Tile's dependency graph, so
                # every producer must be emitted before its consumer and
                # every tile's reads before its pool slot is reused.
                bg_a = deque()   # [key or None, generator]
                done = set()
                bg_b = deque()   # outproj work (plain generators)

                def pump_a(n=1):
                    while n > 0 and bg_a:
                        key, g = bg_a[0]
                        try:
                            next(g)
                            n -= 1
                        except StopIteration:
                            if key is not None:
                                done.add(key)
                            bg_a.popleft()

                def ensure(key):
                    while key not in done:
                        assert bg_a, f"ensure({key}) but queue empty"
                        pump_a(1)

                def pump_b(n=1):
                    while n > 0 and bg_b:
                        try:
                            next(bg_b[0])
                            n -= 1
                        except StopIteration:
                            bg_b.popleft()

                def emit_xdma(b, xdram, sc):
                    xt = xin.tile([P, NKB, SC], bf16, tag="xt")
                    nc.sync.dma_start(
                        xt[:], xmaps[id(xdram)][b, :, :, sc * SC:(sc + 1) * SC])
                    return xt

                def emit_chunk(w_sb, xt, b_sb, dest, sc):
                    """8 accumulating proj MMs + DVE bias-add into dest chunk."""
                    ps = ppm.tile([P, SC], f32, tag="m")
                    for kb in range(NKB):
                        nc.tensor.matmul(
                            ps[:], w_sb[:, kb, :], xt[:, kb, :],
                            start=(kb == 0), stop=(kb == NKB - 1),
                        )
                    nc.vector.tensor_scalar_add(
                        dest[:, sc * SC:(sc + 1) * SC], ps[:], b_sb[:])

                def g_chunk(w_sb, xt, b_sb, dest, sc):
                    ps = ppm.tile([P, SC], f32, tag="m")
                    for kb in range(NKB):
                        nc.tensor.matmul(
                            ps[:], w_sb[:, kb, :], xt[:, kb, :],
                            start=(kb == 0), stop=(kb == NKB - 1),
                        )
                        if kb % 2 == 1 and kb < NKB - 1:
                            yield
                    nc.vector.tensor_scalar_add(
                        dest[:, sc * SC:(sc + 1) * SC], ps[:], b_sb[:])
                    yield

                def emit_transp(v_aug, vT_sb, tb):
                    pst = ppm.tile([P, P], bf16, tag="m")
                    nc.tensor.transpose(
                        pst[:], vT_sb[:, tb * P:(tb + 1) * P], ident[:])
                    nc.vector.tensor_copy(v_aug[:, 0, tb, 0:HD], pst[:, 0:HD])
                    nc.vector.tensor_copy(v_aug[:, 1, tb, 0:HD], pst[:, HD:P])

                def g_transp2(v_aug, vT_sb, tb0):
                    for tb in (tb0, tb0 + 1):
                        emit_transp(v_aug, vT_sb, tb)
                    yield

                def g_norm(oT_sb, o_h0, o_h1, rzs):
                    """bc broadcast MM + psum->sbuf copy + normalize mul,
                    one head per pump; bc tiles alternate psum banks."""
                    for h, o_ps in ((0, o_h0), (1, o_h1)):
                        bc = (ppw if h == 0 else ppm).tile(
                            [HD, SC], f32, tag="w" if h == 0 else "m",
                            name="bc")
                        nc.tensor.matmul(
                            bc[:], ones_row, rzs[h][:], start=True, stop=True)
                        bc_sb = smalls.tile([HD, SC], f32, tag="bcs")
                        nc.vector.tensor_copy(bc_sb[:], bc[:])
                        nc.vector.tensor_mul(
                            oT_sb[h * HD:(h + 1) * HD, :],
                            o_ps[0:HD, :], bc_sb[:],
                        )
                        yield

                def g_outproj(b, sc, oT_sb):
                    ysb = yout.tile([P, 4, D], bf16, tag="y")
                    for sbl in range(4):
                        osl = oT_sb[:, sbl * P:(sbl + 1) * P]
                        for half in range(2):
                            # alternate psum banks (ppw / ppm) so each psy
                            # MM doesn't wait on the previous psy's copy
                            psy = (ppw if half == 0 else ppm).tile(
                                [P, SC], f32, tag="w" if half == 0 else "m",
                                name="psy")
                            nc.tensor.matmul(
                                psy[:], osl,
                                wo_sb[:, half * SC:(half + 1) * SC],
                                start=True, stop=True,
                            )
                            nc.vector.tensor_copy(
                                ysb[:, sbl, half * SC:(half + 1) * SC], psy[:])
                            yield
                    # second HWDGE FIFO (ACT): y-writes don't stall x-loads
                    nc.scalar.dma_start(
                        ypart[b, sc * SC:(sc + 1) * SC, :].rearrange(
                            "(f p) d -> p f d", p=P),
                        ysb[:],
                    )
                    yield

                # --- per-batch state ---
                qT, kT, vT, vaug = {}, {}, {}, {}

                def alloc_batch(b):
                    qT[b] = qkv.tile([P, S], bf16, tag="qT", name=f"qT{b}")
                    kT[b] = qkv.tile([P, S], bf16, tag="kT", name=f"kT{b}")
                    vT[b] = qkv.tile([P, S], bf16, tag="vT", name=f"vT{b}")
                    vaug[b] = vap.tile([P, HPC, NTB, HD + 1], bf16, tag="vaug", name=f"vaug{b}")
                    nc.vector.tensor_copy(
                        vaug[b][:, :, :, HD], ones_f32[:, 0:HPC * NTB])

                # x DMA + chunk consumption order per batch:
                # prologue k0,q0,v0 then k1,v1,q1,k2,v2,k3,v3,q2,q3
                def dma_order(b):
                    o = [(xkT, 0), (xqT, 0), (xvT, 0),
                         (xkT, 1), (xvT, 1), (xqT, 1),
                         (xkT, 2), (xvT, 2), (xkT, 3), (xvT, 3),
                         (xqT, 2), (xqT, 3)]
                    return [(b, t, sc) for (t, sc) in o]

                wmap = {id(xqT): (wq_sb, bq_sb), id(xkT): (wk_sb, bk_sb),
                        id(xvT): (wv_sb, bv_sb)}

                def dest_of(b, t):
                    return {id(xqT): qT[b], id(xkT): kT[b],
                            id(xvT): vT[b]}[id(t)]

                xtiles = {0: {}, 1: {}}

                def tname(t):
                    return {id(xqT): "q", id(xkT): "k", id(xvT): "v"}[id(t)]

                def g_one(fn):
                    fn()
                    yield

                def queue_batch_bg(b, skip_prologue):
                    """Append batch b's x-DMAs + proj + transpose work to
                    bg_a in consumption order.  Each chunk's DMA is queued
                    LOOKAHEAD chunk-units ahead of the chunk itself so at
                    most LOOKAHEAD+1 xin slots are ever live in trace order
                    (xin bufs must cover prologue + lookahead + 1)."""
                    items = dma_order(b)[3:] if skip_prologue else dma_order(b)
                    LOOKAHEAD = 3
                    work = []   # flat list of (key, gen) in consumption order
                    for (bb, t, sc) in items:
                        w_sb, b_sb = wmap[id(t)]
                        work.append((
                            ("dma", b, tname(t), sc),
                            g_one(lambda bb=bb, t=t, sc=sc: xtiles[b].__setitem__(
                                (id(t), sc), emit_xdma(bb, t, sc))),
                        ))
                    # interleave: DMA j stays LOOKAHEAD units ahead of chunk j
                    dmas = work
                    chunks = []
                    for (bb, t, sc) in items:
                        w_sb, b_sb = wmap[id(t)]
                        chunks.append((
                            (tname(t), b, sc),
                            (w_sb, t, b_sb, sc),
                        ))
                        if id(t) == id(xvT):
                            for tb0 in (4 * sc, 4 * sc + 2):
                                chunks.append((("T", b, tb0), (None, None, None, tb0)))
                    di, merged = 0, []
                    for n, (key, spec) in enumerate(chunks):
                        # release DMAs so that chunk consumption never
                        # outruns them and never exceeds the slot budget
                        while di < len(dmas) and di < (
                                sum(1 for k, _ in chunks[:n + 1]
                                    if k[0] != "T") + LOOKAHEAD):
                            merged.append(dmas[di])
                            di += 1
                        merged.append((key, spec))
                    merged.extend(dmas[di:])

                    for key, spec in merged:
                        if key[0] == "dma":
                            bg_a.append((key, spec))
                        elif key[0] == "T":
                            bg_a.append((key, g_transp_lazy(b, key[2])))
                        else:
                            w_sb, t, b_sb, sc = spec
                            bg_a.append((key, g_chunk_lazy(
                                w_sb, b, t, b_sb, sc)))

                def g_chunk_lazy(w_sb, b, t, b_sb, sc):
                    # resolve the x tile at pump time (its DMA item, queued
                    # ahead in bg_a, has already run by ensure-ordering)
                    xt = xtiles[b][(id(t), sc)]
                    yield from g_chunk(w_sb, xt, b_sb, dest_of(b, t), sc)

                def g_transp_lazy(b, tb0):
                    yield from g_transp2(vaug[b], vT[b], tb0)

                def attention(b):
                    """64 pipelined steps (PV skew 2); pumps bg queues in
                    the gaps; ensure() force-drains bg_a for correctness."""
                    norm_pending = []
                    for sc in range(NSC):
                        oT_sb = otp.tile([P, SC], bf16, tag="oT")
                        o_h0 = o_h1 = None
                        ssl = slice(sc * SC, (sc + 1) * SC)
                        exq = deque()
                        ensure(("q", b, sc))
                        for i in range(NTB):
                            tsl = slice(i * P, (i + 1) * P)
                            ensure(("k", b, i // 4))
                            ps_sc = pp.tile([P, 2 * SC], f32, tag="sc")
                            nc.tensor.matmul(
                                ps_sc[:, 0:SC], kT[b][0:HD, tsl],
                                qT[b][0:HD, ssl],
                                start=True, stop=True, tile_position=(0, 0),
                            )
                            if _PROBE != "scores1":
                                nc.tensor.matmul(
                                    ps_sc[:, SC:2 * SC], kT[b][HD:P, tsl],
                                    qT[b][HD:P, ssl],
                                    start=True, stop=True,
                                    tile_position=(64, 0),
                                )
                            ex = expp.tile([P, 2 * SC], bf16, tag="ex")
                            if _PROBE == "noexp":
                                # tiny ACT call keeps structure; PV reads
                                # mostly-garbage ex (timing probe only)
                                nc.scalar.activation(ex[:, 0:P], ps_sc[:, 0:P],
                                                     AF.Exp, scale=0.125)
                            else:
                                nc.scalar.activation(ex[:], ps_sc[:], AF.Exp,
                                                     scale=0.125)
                            exq.append(ex)
                            pump_b(2 if i < 8 else 1)
                            pump_a(1)
                            if len(exq) > 2:
                                pv = i - 2
                                if o_h0 is None:
                                    # drain prev sc's normalize fully before
                                    # reusing its accumulator psum slots
                                    for g in norm_pending:
                                        for _ in g:
                                            pass
                                    norm_pending.clear()
                                    o_h0 = pph.tile([HD + 1, SC], f32,
                                                    tag="oh", name="oh0")
                                    o_h1 = pph.tile([HD + 1, SC], f32,
                                                    tag="oh", name="oh1")
                                pex = exq.popleft()
                                ensure(("T", b, pv - (pv % 2)))
                                nc.tensor.matmul(
                                    o_h0[:], vaug[b][:, 0, pv, :],
                                    pex[:, 0:SC],
                                    start=(pv == 0), stop=False,
                                )
                                nc.tensor.matmul(
                                    o_h1[:], vaug[b][:, 1, pv, :],
                                    pex[:, SC:2 * SC],
                                    start=(pv == 0), stop=False,
                                )
                            if i % 4 == 3:
                                pump_a(1)
                        # drain the pipeline: last two PVs
                        for pv in (NTB - 2, NTB - 1):
                            ensure(("T", b, pv - (pv % 2)))
                            pex = exq.popleft()
                            nc.tensor.matmul(
                                o_h0[:], vaug[b][:, 0, pv, :],
                                pex[:, 0:SC],
                                start=False, stop=(pv == NTB - 1),
                            )
                            nc.tensor.matmul(
                                o_h1[:], vaug[b][:, 1, pv, :],
                                pex[:, SC:2 * SC],
                                start=False, stop=(pv == NTB - 1),
                            )
                        # reciprocals of Z now (DVE); bc/mul pumped during
                        # the next sc's first steps, outproj after
                        rzs = []
                        for h, o_ps in ((0, o_h0), (1, o_h1)):
                            rz = smalls.tile([1, SC], bf16, tag="rz",
                                             name="rz")
                            with nc.allow_low_precision(
                                reason="bf16 1/Z for PE broadcast"
                            ):
                                nc.vector.reciprocal(rz[:], o_ps[HD:HD + 1, :])
                            rzs.append(rz)
                        ng = g_norm(oT_sb, o_h0, o_h1, rzs)
                        bg_b.appendleft(ng)
                        norm_pending.append(ng)
                        bg_b.append(g_outproj(b, sc, oT_sb))
                    # end of batch: drain normalize before next batch's PVs
                    for g in norm_pending:
                        for _ in g:
                            pass

                # ================= emission =================
                for b in range(B):
                    alloc_batch(b)

                # prologue: minimal work for b0-sc0's first steps
                for (t, sc) in ((xkT, 0), (xqT, 0), (xvT, 0)):
                    xtiles[0][(id(t), sc)] = emit_xdma(0, t, sc)
                for (t, sc) in ((xkT, 0), (xqT, 0), (xvT, 0)):
                    w_sb, b_sb = wmap[id(t)]
                    emit_chunk(w_sb, xtiles[0][(id(t), sc)], b_sb,
                               dest_of(0, t), sc)
                for tb in range(4):
                    emit_transp(vaug[0], vT[0], tb)
                for key in (("k", 0, 0), ("q", 0, 0), ("v", 0, 0),
                            ("T", 0, 0), ("T", 0, 2)):
                    done.add(key)

                queue_batch_bg(0, skip_prologue=True)
                queue_batch_bg(1, skip_prologue=False)

                attention(0)
                attention(1)
                # flush remaining background work
                while bg_a:
                    pump_a(64)
                while bg_b:
                    pump_b(64)

            if loop_k == 1:
                body()
            else:
                with tc.For_i(
                    0, loop_k, 1,
                    hint_engines=(
                        mybir.EngineType.PE,
                        mybir.EngineType.DVE,
                        mybir.EngineType.Activation,
                        mybir.EngineType.SP,
                        mybir.EngineType.Pool,
                    ),
                ):
                    body()

    nc.compile()
    _nc_cache[key] = nc
    return nc


def make_in_maps(inputs):
    """Host-side sharding: transpose activations to [B, D, S] bf16, slice
    per-head weights per core."""
    query, key, value = inputs["query"], inputs["key"], inputs["value"]
    Wq, bq, Wk, bk, Wv, bv = (
        inputs["Wq"], inputs["bq"], inputs["Wk"], inputs["bk"],
        inputs["Wv"], inputs["bv"],
    )
    Wo, bo = inputs["Wo"], inputs["bo"]

    xqT = np.ascontiguousarray(np.transpose(query, (0, 2, 1))).astype(_BF16)
    xkT = np.ascontiguousarray(np.transpose(key, (0, 2, 1))).astype(_BF16)
    xvT = np.ascontiguousarray(np.transpose(value, (0, 2, 1))).astype(_BF16)

    in_maps = []
    for c in range(NCORES):
        hs = slice(c * HPC, (c + 1) * HPC)
        # [HPC, HD, D] -> [D, HPC*HD]
        wq_c = np.ascontiguousarray(
            Wq[hs].reshape(HPC * HD, D).T).astype(_BF16)
        wk_c = np.ascontiguousarray(
            Wk[hs].reshape(HPC * HD, D).T).astype(_BF16)
        wv_c = np.ascontiguousarray(
            Wv[hs].reshape(HPC * HD, D).T).astype(_BF16)
        bq_c = np.ascontiguousarray(bq[hs].reshape(P, 1)).astype(np.float32)
        bk_c = np.ascontiguousarray(bk[hs].reshape(P, 1)).astype(np.float32)
        bv_c = np.ascontiguousarray(bv[hs].reshape(P, 1)).astype(np.float32)
        wo_c = np.ascontiguousarray(Wo[:, c * P:(c + 1) * P].T).astype(_BF16)
        in_maps.append({
            "xqT": xqT, "xkT": xkT, "xvT": xvT,
            "wq": wq_c, "wk": wk_c, "wv": wv_c,
            "bq": bq_c, "bk": bk_c, "bv": bv_c,
            "wo": wo_c,
        })
    return in_maps


def make_runner(nc, n_cores=NCORES):
    """Cached jitted shard_map runner (mirrors bass2jax.run_bass_via_pjrt
    without donation so it can be re-invoked for timing)."""
    key = id(nc)
    if key in _runner_cache:
        return _runner_cache[key]
    import jax
    from jax.sharding import Mesh, PartitionSpec
    from jax.experimental.shard_map import shard_map
    import concourse.mybir as mybir
    from concourse import bass2jax

    bass2jax.install_neuronx_cc_hook()
    partition_name = nc.partition_id_tensor.name if nc.partition_id_tensor else None
    in_names, out_names, out_avals = [], [], []
    for alloc in nc.m.functions[0].allocations:
        if not isinstance(alloc, mybir.MemoryLocationSet):
            continue
        name = alloc.memorylocations[0].name
        if alloc.kind == "ExternalInput":
            if name != partition_name:
                in_names.append(name)
        elif alloc.kind == "ExternalOutput":
            out_names.append(name)
            out_avals.append(
                jax.core.ShapedArray(
                    tuple(alloc.tensor_shape), mybir.dt.np(alloc.dtype))
            )
    all_in_names = list(in_names) + ([partition_name] if partition_name else [])

    def _body(*args):
        operands = list(args)
        if partition_name is not None:
            operands.append(bass2jax.partition_id_tensor())
        outs = bass2jax._bass_exec_p.bind(
            *operands, out_avals=tuple(out_avals),
            in_names=tuple(all_in_names), out_names=tuple(out_names),
            lowering_input_output_aliases=(),
            sim_require_finite=False, sim_require_nnan=False, nc=nc)
        return tuple(outs)

    devices = jax.devices()[:n_cores]
    mesh = Mesh(np.asarray(devices), ("core",))
    fn = jax.jit(shard_map(
        _body, mesh=mesh,
        in_specs=(PartitionSpec("core"),) * len(in_names),
        out_specs=(PartitionSpec("core"),) * len(out_names),
        check_rep=False))
    out = (fn, in_names, out_names, out_avals)
    _runner_cache[key] = out
    return out


def run_on_cores(nc, in_maps):
    """Run the module on the 8 cores; returns list of per-core out dicts."""
    import jax
    fn, in_names, out_names, out_avals = make_runner(nc)
    concat_in = [
        np.concatenate([m[nm] for m in in_maps], axis=0) for nm in in_names
    ]
    outs = jax.block_until_ready(fn(*concat_in))
    res = []
    for c in range(len(in_maps)):
        d = {}
        for i, nm in enumerate(out_names):
            shp = out_avals[i].shape
            d[nm] = np.asarray(outs[i]).reshape(len(in_maps), *shp)[c]
        res.append(d)
    return res


def postprocess(results, inputs):
    """Sum per-core partial outputs and add the output bias."""
    acc = np.zeros((B, S, D), dtype=np.float64)
    for r in results:
        acc += r["ypart"].astype(np.float64)
    acc += inputs["bo"].astype(np.float64)
    return acc.astype(np.float32)


def kernel(**inputs) -> np.ndarray:
    inputs = {k: np.asarray(v) for k, v in inputs.items()}
    nc = build_nc(loop_k=1)
    in_maps = make_in_maps(inputs)
    results = run_on_cores(nc, in_maps)
    return postprocess(results, inputs)
